# revision 2
# baseline (speedup 1.0000x reference)
"""Trainium2 Bass kernel for nn_HamiltonianVersorNN.

Math: the reference energy reads only blade-0 of the final layer, and the
versor gate h*sigmoid(h[...,0:1]) makes blade-0 evolve as elementwise SiLU.
Backprop therefore collapses exactly to a 2-layer SiLU MLP on blade-0:

    a1 = A x + c1            A  = W1 @ W_in[:, ::32].T          [32, 6]
    a2 = W2 silu(a1) + c2    c1 = W1 @ b_in[::32] + b1[:, 0]
    dx = A.T (W2.T (w3 * silu'(a2)) * silu'(a1))
    out = x + dt * [dx[3:6], -dx[0:3]]

Sharding: pure data parallel over B*S*N positions, 8 cores, 16384
positions/core. On-chip layout packs 4 tokens per 128-partition column
(partition 32*tl + c holds channel c of token 4g+tl) so the 32x32
channel-mix matmuls contract over the full 128 partitions via
block-diagonal stationaries.

Perf design (v2, prior 40.9us -> target ~18us):
- The v1 kernel was PE-bound: f32r movings silently ran in fp32-HIGH
  4-pass mode (~790ns/512col) AND disabled fast-weight-load for the
  following bf16 matmuls; 5 passes x 4096 cols = ~32us of PE time at
  ~1.2GHz effective. v2 streams EVERYTHING through the PE as fp16
  (x, stationaries, and the ACT/DVE-produced movings), which keeps FWL
  on and the per-matmul cost at the warm production rate.
- a2 = W2 silu(a1) + c2 is split as (W2/2)a1 + (W2/2)(a1 tanh(a1/2)):
  the first term composes with the input layer ((W2 A/2) x) and streams
  straight from x; the only layer-1 elementwise product is w = a1*tau1
  (DVE). Tanh and Derivative_silu share one ACT table set
  (derivative_silu_and_others) so there are zero table switches.
- Residual rides the PE: po = Bout g1 + I x accumulated in PSUM (x
  moving in fp16: residual error ~2^-11, far inside the 2e-2 gate).
  DMA cannot read PSUM, so po chunks are quadrant-packed into partition
  strips (512-col chunk h -> partitions 32h..32h+24) and the mandatory
  PSUM->SBUF evacuation is a single DVE copy whose cost scales with the
  (halved) free size, not the partition count.
- Engine budget per core (4096 cols): ACT 3 passes (tau, d1, d2)
  ~13-14us = bottleneck; DVE w + g1 + evac ~12.5us; PE 6 fp16 passes
  (l1, l2x, l2w, l3, l4, res) ~7-10us warm; const DMAs ride the GpSimd
  queue (NOT scalar - the Scalar queue is the bottleneck engine), x
  slabs are all prefetched at t0 on Sync, outputs ride Sync.
- Pipelining: back half (d2/l3/g1/l4+res/evac/DMA) runs one block
  behind the front half (l1/l2x/tau/d1/w/l2w); two small 256-col drain
  blocks shorten the final serial tail. PSUM: a1 double-buffered
  [2x2 banks] and a double-buffered mid arena [2x2 banks] through which
  a2 -> v1 -> po rotate (sequential lifetimes within a block).
"""

import sys

import numpy as np

if "/opt/trn_rl_repo" not in sys.path:
    sys.path.insert(0, "/opt/trn_rl_repo")

import concourse.bass as bass
import concourse.tile as tile
from concourse import mybir

AF = mybir.ActivationFunctionType
F32 = mybir.dt.float32
F16 = mybir.dt.float16

N_CORES = 8
B, S, N, D = 32, 256, 16, 6
HIDDEN = 32
BLADES = 32
DT = 0.01

TOK_TOTAL = B * S * N          # 131072 positions
TOK_CORE = TOK_TOTAL // N_CORES  # 16384
TPC = 4                        # tokens packed per 128-partition column
GROUPS = TOK_CORE // TPC       # 4096 columns per core
MM = 512                       # matmul free-dim (1 PSUM bank fp32)
BD = 1024                      # max block free-dim (2 PSUM banks)
BLOCKS = [(0, 512), (512, 1024), (1536, 1024), (2560, 1024),
          (3584, 256), (3840, 256)]
OUTW = 512                     # per-block HBM stride of the packed output

KP = TPC * D                   # 24 partitions of x / out rows
KPI = KP + 1                   # + constant ones row carrying the biases


def _chunks(wd):
    """512-col matmul chunks of a block: (h, off, width)."""
    out = []
    h = 0
    off = 0
    while off < wd:
        out.append((h, off, min(MM, wd - off)))
        h += 1
        off += MM
    return out


def _build_nc():
    nc = bass.Bass()

    xg = nc.dram_tensor("xg", [KPI, GROUPS], F16, kind="ExternalInput")
    l12 = nc.dram_tensor("l12", [KPI, 256], F16, kind="ExternalInput")
    l2w = nc.dram_tensor("l2w", [128, 128], F16, kind="ExternalInput")
    l3 = nc.dram_tensor("l3", [128, 128], F16, kind="ExternalInput")
    l4 = nc.dram_tensor("l4", [128, KP], F16, kind="ExternalInput")
    resi = nc.dram_tensor("resi", [KPI, KP], F16, kind="ExternalInput")
    outg = nc.dram_tensor("outg", [64, OUTW * len(BLOCKS)], F32,
                          kind="ExternalOutput")

    with tile.TileContext(nc) as tc:
        with (
            tc.tile_pool(name="consts", bufs=1) as consts,
            tc.tile_pool(name="xin", bufs=len(BLOCKS)) as xin,
            tc.tile_pool(name="work", bufs=3) as work,
            tc.tile_pool(name="psA", bufs=2, space="PSUM") as psA,
            tc.tile_pool(name="psB", bufs=2, space="PSUM") as psB,
        ):
            # Const loads on the GpSimd DGE queue: Sync is reserved for the
            # x slabs (so block 0's input lands first) and Scalar is the
            # bottleneck compute engine - a DMA there costs ~667ns of ACT
            # sequencer time.
            sb_l12 = consts.tile([KPI, 256], F16)
            nc.gpsimd.dma_start(out=sb_l12[:], in_=l12[:])
            sb_l1 = sb_l12[:, 0:128]
            sb_l2x = sb_l12[:, 128:256]
            sb_l2w = consts.tile([128, 128], F16)
            nc.gpsimd.dma_start(out=sb_l2w[:], in_=l2w[:])
            sb_l3 = consts.tile([128, 128], F16)
            nc.gpsimd.dma_start(out=sb_l3[:], in_=l3[:])
            sb_l4 = consts.tile([128, KP], F16)
            nc.gpsimd.dma_start(out=sb_l4[:], in_=l4[:])
            sb_resi = consts.tile([KPI, KP], F16)
            nc.gpsimd.dma_start(out=sb_resi[:], in_=resi[:])

            # Dummy first activation: walrus attaches the ACT table load to
            # the first Activation instruction, which can then carry only a
            # single sync wait. Give it a single-wait warm-up op.
            warm = consts.tile([1, 128], F32)
            nc.vector.memset(warm[:], 0.0)
            nc.scalar.activation(warm[:], warm[:], AF.Derivative_silu)

            # Prefetch ALL x slabs at t0 on the Sync HWDGE queue; nothing
            # ever overwrites them (bufs = n_blocks), so no WAR hazards.
            sb_xs = []
            for bi, (c0, wd) in enumerate(BLOCKS):
                sb_x = xin.tile([KPI, wd], F16, tag=f"x_{bi}")
                nc.sync.dma_start(out=sb_x[:], in_=xg[:, c0 : c0 + wd])
                sb_xs.append(sb_x)

            def back_half(st):
                """d2 -> v1 -> g1 -> po(+res) -> evac -> DMA for a block."""
                bi, wd, mid, d1, sb_x = st
                ws = slice(0, wd)
                # d2 = silu'(a2)
                d2 = work.tile([128, wd], F16, tag=f"d2_{wd}")
                nc.scalar.activation(d2[:], mid[:, ws], AF.Derivative_silu)

                # v1 = blockdiag(diag(w3) W2)^T @ d2, overwriting a2
                for h, off, cw in _chunks(wd):
                    ms = slice(off, off + cw)
                    nc.tensor.matmul(mid[:, ms], sb_l3[:], d2[:, ms],
                                     start=True, stop=True)

                # g1 = v1 * d1
                g1 = work.tile([128, wd], F16, tag=f"g1_{wd}")
                nc.vector.tensor_mul(g1[:], mid[:, ws], d1[:, ws])

                # po = blockdiag(Bout) @ g1 + I @ x, quadrant-packed:
                # chunk h lands on partitions 32h..32h+24, cols 0..cw.
                # Overwrites v1 (dead after g1) at the head of mid.
                nch = 0
                for h, off, cw in _chunks(wd):
                    po = mid[32 * h : 32 * h + KP, 0:cw]
                    nc.tensor.matmul(po, sb_l4[:], g1[:, off : off + cw],
                                     start=True, stop=False)
                    nc.tensor.matmul(po, sb_resi[:], sb_x[:, off : off + cw],
                                     start=False, stop=True)
                    nch += 1

                # Mandatory PSUM->SBUF evacuation (DMA has no PSUM route);
                # the quadrant packing keeps its free-size at <=512.
                ew = min(wd, MM)
                sb_o = work.tile([32 * nch, ew], F32, tag=f"o_{wd}")
                nc.vector.tensor_copy(sb_o[:], mid[0 : 32 * nch, 0:ew])
                nc.sync.dma_start(
                    out=outg[0 : 32 * nch, OUTW * bi : OUTW * bi + ew],
                    in_=sb_o[:],
                )

            pending = None
            for bi, (c0, wd) in enumerate(BLOCKS):
                sb_x = sb_xs[bi]
                ws = slice(0, wd)

                # a1 = blockdiag(A) @ x + c1 (c1 rides the ones row)
                a1 = psA.tile([128, BD], F32, tag="a1")
                for h, off, cw in _chunks(wd):
                    ms = slice(off, off + cw)
                    nc.tensor.matmul(a1[:, ms], sb_l1, sb_x[:, ms],
                                     start=True, stop=True)

                # a2 = W2 silu(a1) + c2 split as (W2/2) a1 + (W2/2)(a1*tau):
                # the first term composes with the input layer, so it streams
                # straight from x (l2x = blockdiag(W2 A / 2) with bias row
                # W2 c1 / 2 + c2).
                mid = psB.tile([128, BD], F32, tag="mid")
                for h, off, cw in _chunks(wd):
                    ms = slice(off, off + cw)
                    nc.tensor.matmul(mid[:, ms], sb_l2x, sb_x[:, ms],
                                     start=True, stop=False)

                # tau = tanh(0.5*a1)
                tau = work.tile([128, wd], F16, tag=f"tau_{wd}")
                nc.scalar.activation(tau[:], a1[:, ws], AF.Tanh, scale=0.5)

                # w = a1 * tau (the only layer-1 elementwise product);
                # issued before the previous block's back half so it leads
                # the DVE queue.
                w = work.tile([128, wd], F16, tag=f"w_{wd}")
                nc.vector.tensor_mul(w[:], a1[:, ws], tau[:])

                # d1 = silu'(a1) (consumer g1 is a block away)
                d1 = work.tile([128, wd], F16, tag=f"d1_{wd}")
                nc.scalar.activation(d1[:], a1[:, ws], AF.Derivative_silu)

                # Previous block's back half: its d2 follows tau(k) on ACT;
                # its matmuls precede the w-dependent l2w below on PE.
                if pending is not None:
                    back_half(pending)

                # a2 += blockdiag(W2/2) @ w
                for h, off, cw in _chunks(wd):
                    ms = slice(off, off + cw)
                    nc.tensor.matmul(mid[:, ms], sb_l2w[:], w[:, ms],
                                     start=False, stop=True)

                pending = (bi, wd, mid, d1, sb_x)

            back_half(pending)

    return nc


def _split_multi_waits(nc):
    """This walrus build rejects engine instructions carrying more than one
    sync wait ("Too many sync wait commands"). Hoist all but one wait of
    each instruction onto standalone NoOps issued just before it on the
    same engine (engines execute their queue in order, so semantics are
    preserved)."""
    for f in nc.m.functions:
        for b in f.blocks:
            insts = list(b.instructions)
            out = []
            changed = False
            for inst in insts:
                # This walrus build also rejects the raw-ISA
                # EVENT_SEMAPHORE_RANGE_CLEAR Tile emits at context end
                # ("ISA wrong length" - ISA table version skew). The NEFF
                # preamble re-initializes semaphores, so drop it.
                if (
                    type(inst).__name__ == "InstISA"
                    and getattr(inst, "op_name", "") == "EVENT_SEMAPHORE_RANGE_CLEAR"
                ):
                    changed = True
                    continue
                si = getattr(inst, "sync_info", None)
                waits = list(si.on_wait) if si is not None and si.on_wait else []
                if len(waits) > 1:
                    changed = True
                    for k, w in enumerate(waits[:-1]):
                        nop = mybir.InstNoOp(name=f"{inst.name}-w{k}", ins=[], outs=[])
                        nop.engine = inst.engine
                        nop.sync_info = mybir.SyncInfo(on_wait=[w], on_update=[])
                        out.append(nop)
                    inst.sync_info = mybir.SyncInfo(
                        on_wait=[waits[-1]], on_update=list(si.on_update or [])
                    )
                out.append(inst)
            if changed:
                b.instructions = out
    return nc


_NC_CACHE = None


def _get_nc():
    global _NC_CACHE
    if _NC_CACHE is None:
        _NC_CACHE = _split_multi_waits(_build_nc())
    return _NC_CACHE


def _prep_weights(W_in, b_in, W1, b1, W2, b2, W3, b3):
    """Host-side constant folding into the kernel's stationary layouts."""
    W_in = np.asarray(W_in, np.float64)
    b_in = np.asarray(b_in, np.float64)
    W1 = np.asarray(W1, np.float64)
    b1 = np.asarray(b1, np.float64)
    W2 = np.asarray(W2, np.float64)
    b2 = np.asarray(b2, np.float64)
    W3 = np.asarray(W3, np.float64)

    Win0 = W_in[:, ::BLADES]            # [6, 8]
    bin0 = b_in[::BLADES]               # [8]
    A = W1 @ Win0.T                     # [32, 6]
    c1 = W1 @ bin0 + b1[:, 0]           # [32]
    c2 = b2[:, 0]                       # [32]
    w3 = W3[0, :]                       # [32]

    # Bout[d, c]: out[d] += dt*dx[d+3] (d<3), -dt*dx[d-3] (d>=3); dx = A^T g1
    Bout = np.zeros((D, HIDDEN))
    Bout[0:3, :] = DT * A[:, 3:6].T
    Bout[3:6, :] = -DT * A[:, 0:3].T

    # a2 = W2 silu(a1) + c2 = (W2/2) a1 + (W2/2)(a1 tau1) + c2, and
    # (W2/2) a1 = (W2 A / 2) x + W2 c1 / 2  composes with the input layer.
    A2x = 0.5 * W2 @ A                  # [32, 6]
    c2x = 0.5 * W2 @ c1 + c2            # [32]

    l12 = np.zeros((KPI, 256), np.float16)
    l1 = l12[:, 0:128]
    l2xm = l12[:, 128:256]
    l2wm = np.zeros((128, 128), np.float16)
    l3 = np.zeros((128, 128), np.float16)
    l4 = np.zeros((128, KP), np.float16)
    resi = np.zeros((KPI, KP), np.float16)
    for tl in range(TPC):
        # l1[6tl+d, 32tl+c] = A[c, d]; l1[24, 32tl+c] = c1[c]
        l1[6 * tl : 6 * tl + 6, 32 * tl : 32 * tl + 32] = A.T.astype(np.float16)
        l1[KP, 32 * tl : 32 * tl + 32] = c1.astype(np.float16)
        # l2x[6tl+d, 32tl+c] = A2x[c, d]; ones row carries c2x
        l2xm[6 * tl : 6 * tl + 6, 32 * tl : 32 * tl + 32] = A2x.T.astype(
            np.float16
        )
        l2xm[KP, 32 * tl : 32 * tl + 32] = c2x.astype(np.float16)
        # l2w[32tl+ci, 32tl+co] = W2[co, ci] / 2
        l2wm[32 * tl : 32 * tl + 32, 32 * tl : 32 * tl + 32] = (
            0.5 * W2.T
        ).astype(np.float16)
        # l3[32tl+co, 32tl+ci] = w3[co] * W2[co, ci]
        l3[32 * tl : 32 * tl + 32, 32 * tl : 32 * tl + 32] = (
            w3[:, None] * W2
        ).astype(np.float16)
        # l4[32tl+c, 6tl+d] = Bout[d, c]
        l4[32 * tl : 32 * tl + 32, 6 * tl : 6 * tl + 6] = Bout.T.astype(
            np.float16
        )
        # residual identity: po[6tl+d] += x[6tl+d]; ones row contributes 0
        for d in range(D):
            resi[6 * tl + d, 6 * tl + d] = np.float16(1.0)

    return {
        "l12": l12,
        "l2w": l2wm,
        "l3": l3,
        "l4": l4,
        "resi": resi,
    }


def _shard_x(x):
    """[B,S,N,D] -> list of per-core fp16 [25, GROUPS] arrays (row 24 = 1)."""
    xf = np.ascontiguousarray(np.asarray(x, np.float32)).reshape(TOK_TOTAL, D)
    shards = []
    for c in range(N_CORES):
        xc = xf[c * TOK_CORE : (c + 1) * TOK_CORE]          # [16384, 6]
        xgc = np.empty((KPI, GROUPS), np.float16)
        xgc[:KP] = (
            xc.reshape(GROUPS, TPC, D).transpose(1, 2, 0).reshape(KP, GROUPS)
        )
        xgc[KP] = np.float16(1.0)
        shards.append(xgc)
    return shards


def _unshard_out(outs):
    """list of per-core [64, OUTW*nblocks] -> [B,S,N,D].

    Block bi covers global cols c0..c0+wd; its 512-col chunk h sits on
    partitions 32h..32h+24 of outg[:, OUTW*bi : OUTW*bi+cw]."""
    full = np.empty((TOK_TOTAL, D), np.float32)
    for c, og in enumerate(outs):
        og = np.asarray(og)
        oc = np.empty((KP, GROUPS), np.float32)
        for bi, (c0, wd) in enumerate(BLOCKS):
            for h, off, cw in _chunks(wd):
                oc[:, c0 + off : c0 + off + cw] = og[
                    32 * h : 32 * h + KP, OUTW * bi : OUTW * bi + cw
                ]
        occ = oc.reshape(TPC, D, GROUPS).transpose(2, 0, 1).reshape(TOK_CORE, D)
        full[c * TOK_CORE : (c + 1) * TOK_CORE] = occ
    return full.reshape(B, S, N, D)


# Test-harness knobs (ignored in normal use): set kernel._TRACE = True to
# collect an NTFF profile; the BassKernelResults lands in kernel._LAST_RES.
_TRACE = False
_LAST_RES = None


def kernel(x, W_in, b_in, W1, b1, W2, b2, W3, b3):
    global _LAST_RES
    from concourse.bass_utils import run_bass_kernel_spmd

    nc = _get_nc()
    consts = _prep_weights(W_in, b_in, W1, b1, W2, b2, W3, b3)
    shards = _shard_x(x)
    in_maps = [{"xg": shards[c], **consts} for c in range(N_CORES)]
    res = run_bass_kernel_spmd(nc, in_maps, list(range(N_CORES)), trace=_TRACE)
    _LAST_RES = res
    return _unshard_out([res.results[c]["outg"] for c in range(N_CORES)])


# revision 4
# speedup vs baseline: 1.0283x; 1.0283x over previous
"""Trainium2 Bass kernel for nn_HamiltonianVersorNN.

Math: the reference energy reads only blade-0 of the final layer, and the
versor gate h*sigmoid(h[...,0:1]) makes blade-0 evolve as elementwise SiLU.
Backprop therefore collapses exactly to a 2-layer SiLU MLP on blade-0:

    a1 = A x + c1            A  = W1 @ W_in[:, ::32].T          [32, 6]
    a2 = W2 silu(a1) + c2    c1 = W1 @ b_in[::32] + b1[:, 0]
    dx = A.T (W2.T (w3 * silu'(a2)) * silu'(a1))
    out = x + dt * [dx[3:6], -dx[0:3]]

Sharding: pure data parallel over B*S*N positions, 8 cores, 16384
positions/core. On-chip layout packs 4 tokens per 128-partition column
(partition 32*tl + c holds channel c of token 4g+tl) so the 32x32
channel-mix matmuls contract over the full 128 partitions via
block-diagonal stationaries.

Perf design (v3, prior 40.9us):
- The v1 kernel was PE-bound: f32r movings silently ran in fp32-HIGH
  4-pass mode AND disabled fast-weight-load for the following bf16
  matmuls. The PE streams at ~1.2 GHz on this part (512-col matmul =
  ~427ns), so matmul passes over the 4096 columns dominate. v3 runs
  everything through the PE as fp16 and cuts the pass count to 4:
  l1 (a1 from x), l2x (a2 partial from x), l2w (a2 from w), l3 (v1
  from d2), l4 (po from g1).
- a2 = W2 silu(a1) + c2 is split as (W2/2)a1 + (W2/2)(a1 tanh(a1/2)):
  the first term composes with the input layer ((W2 A/2) x) and streams
  straight from x; the only layer-1 elementwise product is w = a1*tau1
  (DVE). Tanh and Derivative_silu share one ACT table set
  (derivative_silu_and_others) so there are zero table switches.
- The residual rides the mandatory PSUM->SBUF evacuation (DMA has no
  PSUM route): out = po + x as a DVE tensor_add against a host-prepared
  quadrant-packed fp32 copy of x, so the residual costs nothing extra
  and stays full precision. po chunks are quadrant-packed by the l4
  matmuls into partition strips (512-col chunk h -> partitions
  32h..32h+24), halving the evacuation's free-size on 1024 blocks.
- Engine budget per core (4096 cols): ACT 3 passes (tau, d1, d2)
  ~15us; PE 5 fp16 passes ~17us incl ldweights; DVE w + g1 + evac-add
  ~13us. Const DMAs ride the Vector queue, x and outputs ride Sync,
  GpSimd runs no SWDGE at all (slow teardown quiesce).
- Pipelining: back half split in two: (d2, l3) issue right after tau(k)
  so d2(k-1) takes the second ACT slot and l3(k-1) does not block the
  w-dependent l2w(k) on the PE queue; (g1, l4, evac, DMA) issue after
  l2w(k). Two small 256-col drain blocks shorten the final serial tail.
  PSUM: a1 double-buffered [2x2 banks], mid arena double-buffered
  [2x2 banks] rotating a2 -> v1 -> po.
"""

import sys

import numpy as np

if "/opt/trn_rl_repo" not in sys.path:
    sys.path.insert(0, "/opt/trn_rl_repo")

import concourse.bass as bass
import concourse.tile as tile
from concourse import mybir

AF = mybir.ActivationFunctionType
F32 = mybir.dt.float32
F16 = mybir.dt.float16

N_CORES = 8
B, S, N, D = 32, 256, 16, 6
HIDDEN = 32
BLADES = 32
DT = 0.01

TOK_TOTAL = B * S * N          # 131072 positions
TOK_CORE = TOK_TOTAL // N_CORES  # 16384
TPC = 4                        # tokens packed per 128-partition column
GROUPS = TOK_CORE // TPC       # 4096 columns per core
MM = 512                       # matmul free-dim (1 PSUM bank fp32)
BD = 1024                      # max block free-dim (2 PSUM banks)
BLOCKS = [(0, 512), (512, 1024), (1536, 1024), (2560, 1024),
          (3584, 256), (3840, 256)]
OUTW = 512                     # per-block HBM stride of the packed output
NB = len(BLOCKS)

KP = TPC * D                   # 24 partitions of x / out rows
KPI = KP + 1                   # + constant ones row carrying the biases


def _chunks(wd):
    """512-col matmul chunks of a block: (h, off, width)."""
    out = []
    h = 0
    off = 0
    while off < wd:
        out.append((h, off, min(MM, wd - off)))
        h += 1
        off += MM
    return out


def _build_nc():
    nc = bass.Bass()

    xg = nc.dram_tensor("xg", [KPI, GROUPS], F16, kind="ExternalInput")
    xq = nc.dram_tensor("xq", [64, OUTW * NB], F32, kind="ExternalInput")
    l12 = nc.dram_tensor("l12", [KPI, 256], F16, kind="ExternalInput")
    l2w = nc.dram_tensor("l2w", [128, 128], F16, kind="ExternalInput")
    l3 = nc.dram_tensor("l3", [128, 128], F16, kind="ExternalInput")
    l4 = nc.dram_tensor("l4", [128, KP], F16, kind="ExternalInput")
    outg = nc.dram_tensor("outg", [64, OUTW * NB], F32, kind="ExternalOutput")

    with tile.TileContext(nc) as tc:
        with (
            tc.tile_pool(name="consts", bufs=1) as consts,
            tc.tile_pool(name="xin", bufs=1) as xin,
            tc.tile_pool(name="work", bufs=3) as work,
            tc.tile_pool(name="psA", bufs=2, space="PSUM") as psA,
            tc.tile_pool(name="psB", bufs=2, space="PSUM") as psB,
        ):
            # l12 gates the very first matmul, so it leads the Sync queue
            # (ahead of the x slabs). The back-half stationaries ride the
            # GpSimd SWDGE queue - they are only needed ~2 blocks in, and
            # early const loads quiesce long before teardown (the slow-
            # quiesce concern applies to late OUTPUT DMAs). Scalar stays
            # clean for the ACT table load.
            sb_l12 = consts.tile([KPI, 256], F16)
            nc.sync.dma_start(out=sb_l12[:], in_=l12[:])
            sb_l1 = sb_l12[:, 0:128]
            sb_l2x = sb_l12[:, 128:256]
            sb_l2w = consts.tile([128, 128], F16)
            nc.gpsimd.dma_start(out=sb_l2w[:], in_=l2w[:])
            sb_l3 = consts.tile([128, 128], F16)
            nc.gpsimd.dma_start(out=sb_l3[:], in_=l3[:])
            sb_l4 = consts.tile([128, KP], F16)
            nc.gpsimd.dma_start(out=sb_l4[:], in_=l4[:])

            # x arrives in two slabs (block 0's lands first), the
            # quadrant-packed fp32 residual copy in one.
            W0 = BLOCKS[0][1]
            sb_x0 = xin.tile([KPI, W0], F16)
            nc.sync.dma_start(out=sb_x0[:], in_=xg[:, 0:W0])
            sb_xr = xin.tile([KPI, GROUPS - W0], F16)
            nc.sync.dma_start(out=sb_xr[:], in_=xg[:, W0:GROUPS])
            sb_xq = xin.tile([64, OUTW * NB], F32)
            nc.sync.dma_start(out=sb_xq[:], in_=xq[:])

            def xslab(bi, lo, hi):
                """SBUF view of x columns [lo, hi) of block bi."""
                c0 = BLOCKS[bi][0]
                if bi == 0:
                    return sb_x0[:, c0 + lo : c0 + hi]
                return sb_xr[:, c0 - W0 + lo : c0 - W0 + hi]

            # Dummy first activation: walrus attaches the ACT table load to
            # the first Activation instruction, which can then carry only a
            # single sync wait. Give it a single-wait warm-up op.
            warm = consts.tile([1, 128], F32)
            nc.vector.memset(warm[:], 0.0)
            nc.scalar.activation(warm[:], warm[:], AF.Derivative_silu)

            def back_early(st):
                """d2 -> v1 for the previous block. Issued right after
                tau(k) so d2(k-1) - whose input has been ready since last
                block - takes the second ACT slot, and l3(k-1) sits ahead
                of the w-dependent l2w(k) on the PE queue."""
                bi, wd, mid, d1 = st
                d2 = work.tile([128, wd], F16, tag=f"d2_{wd}")
                nc.scalar.activation(d2[:], mid[:, 0:wd], AF.Derivative_silu)
                for h, off, cw in _chunks(wd):
                    ms = slice(off, off + cw)
                    nc.tensor.matmul(mid[:, ms], sb_l3[:], d2[:, ms],
                                     start=True, stop=True)
                return d2

            def back_late(st):
                """g1 -> po (quadrant-packed) -> evac(+residual) -> DMA."""
                bi, wd, mid, d1 = st
                g1 = work.tile([128, wd], F16, tag=f"g1_{wd}")
                nc.vector.tensor_mul(g1[:], mid[:, 0:wd], d1[:, 0:wd])

                # po = blockdiag(Bout) @ g1, chunk h on partitions
                # 32h..32h+24 (overwrites v1, dead after g1).
                nch = 0
                for h, off, cw in _chunks(wd):
                    po = mid[32 * h : 32 * h + KP, 0:cw]
                    nc.tensor.matmul(po, sb_l4[:], g1[:, off : off + cw],
                                     start=True, stop=True)
                    nch += 1

                # Mandatory PSUM->SBUF evacuation doubles as the residual
                # add against the quadrant-packed fp32 x.
                ew = min(wd, MM)
                sb_o = work.tile([32 * nch, ew], F32, tag=f"o_{wd}")
                nc.vector.tensor_add(
                    sb_o[:],
                    mid[0 : 32 * nch, 0:ew],
                    sb_xq[0 : 32 * nch, OUTW * bi : OUTW * bi + ew],
                )
                nc.sync.dma_start(
                    out=outg[0 : 32 * nch, OUTW * bi : OUTW * bi + ew],
                    in_=sb_o[:],
                )

            pending = None
            for bi, (c0, wd) in enumerate(BLOCKS):
                # a1 = blockdiag(A) @ x + c1 (c1 rides the ones row)
                a1 = psA.tile([128, BD], F32, tag="a1")
                for h, off, cw in _chunks(wd):
                    nc.tensor.matmul(a1[:, off : off + cw], sb_l1,
                                     xslab(bi, off, off + cw),
                                     start=True, stop=True)

                # a2 = W2 silu(a1) + c2 split as (W2/2) a1 + (W2/2)(a1*tau):
                # the first term composes with the input layer, so it
                # streams straight from x (l2x = blockdiag(W2 A / 2) with
                # bias row W2 c1 / 2 + c2).
                mid = psB.tile([128, BD], F32, tag="mid")
                for h, off, cw in _chunks(wd):
                    nc.tensor.matmul(mid[:, off : off + cw], sb_l2x,
                                     xslab(bi, off, off + cw),
                                     start=True, stop=False)

                # tau = tanh(0.5*a1)
                tau = work.tile([128, wd], F16, tag=f"tau_{wd}")
                nc.scalar.activation(tau[:], a1[:, 0:wd], AF.Tanh, scale=0.5)

                # w = a1 * tau; issued now so it leads the DVE queue.
                w = work.tile([128, wd], F16, tag=f"w_{wd}")
                nc.vector.tensor_mul(w[:], a1[:, 0:wd], tau[:])

                if pending is not None:
                    back_early(pending)

                # d1 = silu'(a1) (consumer g1 is a block away, so d2(k-1)
                # above takes the second ACT slot)
                d1 = work.tile([128, wd], F16, tag=f"d1_{wd}")
                nc.scalar.activation(d1[:], a1[:, 0:wd], AF.Derivative_silu)

                # a2 += blockdiag(W2/2) @ w
                for h, off, cw in _chunks(wd):
                    nc.tensor.matmul(mid[:, off : off + cw], sb_l2w[:],
                                     w[:, off : off + cw],
                                     start=False, stop=True)

                if pending is not None:
                    back_late(pending)

                pending = (bi, wd, mid, d1)

            back_early(pending)
            back_late(pending)

    return nc


def _split_multi_waits(nc):
    """This walrus build rejects engine instructions carrying more than one
    sync wait ("Too many sync wait commands"). Hoist all but one wait of
    each instruction onto standalone NoOps issued just before it on the
    same engine (engines execute their queue in order, so semantics are
    preserved)."""
    for f in nc.m.functions:
        for b in f.blocks:
            insts = list(b.instructions)
            out = []
            changed = False
            for inst in insts:
                # This walrus build also rejects the raw-ISA
                # EVENT_SEMAPHORE_RANGE_CLEAR Tile emits at context end
                # ("ISA wrong length" - ISA table version skew). The NEFF
                # preamble re-initializes semaphores, so drop it.
                if (
                    type(inst).__name__ == "InstISA"
                    and getattr(inst, "op_name", "") == "EVENT_SEMAPHORE_RANGE_CLEAR"
                ):
                    changed = True
                    continue
                si = getattr(inst, "sync_info", None)
                waits = list(si.on_wait) if si is not None and si.on_wait else []
                if len(waits) > 1:
                    changed = True
                    for k, w in enumerate(waits[:-1]):
                        nop = mybir.InstNoOp(name=f"{inst.name}-w{k}", ins=[], outs=[])
                        nop.engine = inst.engine
                        nop.sync_info = mybir.SyncInfo(on_wait=[w], on_update=[])
                        out.append(nop)
                    inst.sync_info = mybir.SyncInfo(
                        on_wait=[waits[-1]], on_update=list(si.on_update or [])
                    )
                out.append(inst)
            if changed:
                b.instructions = out
    return nc


_NC_CACHE = None


def _get_nc():
    global _NC_CACHE
    if _NC_CACHE is None:
        _NC_CACHE = _split_multi_waits(_build_nc())
    return _NC_CACHE


def _prep_weights(W_in, b_in, W1, b1, W2, b2, W3, b3):
    """Host-side constant folding into the kernel's stationary layouts."""
    W_in = np.asarray(W_in, np.float64)
    b_in = np.asarray(b_in, np.float64)
    W1 = np.asarray(W1, np.float64)
    b1 = np.asarray(b1, np.float64)
    W2 = np.asarray(W2, np.float64)
    b2 = np.asarray(b2, np.float64)
    W3 = np.asarray(W3, np.float64)

    Win0 = W_in[:, ::BLADES]            # [6, 8]
    bin0 = b_in[::BLADES]               # [8]
    A = W1 @ Win0.T                     # [32, 6]
    c1 = W1 @ bin0 + b1[:, 0]           # [32]
    c2 = b2[:, 0]                       # [32]
    w3 = W3[0, :]                       # [32]

    # Bout[d, c]: out[d] += dt*dx[d+3] (d<3), -dt*dx[d-3] (d>=3); dx = A^T g1
    Bout = np.zeros((D, HIDDEN))
    Bout[0:3, :] = DT * A[:, 3:6].T
    Bout[3:6, :] = -DT * A[:, 0:3].T

    # a2 = W2 silu(a1) + c2 = (W2/2) a1 + (W2/2)(a1 tau1) + c2, and
    # (W2/2) a1 = (W2 A / 2) x + W2 c1 / 2  composes with the input layer.
    A2x = 0.5 * W2 @ A                  # [32, 6]
    c2x = 0.5 * W2 @ c1 + c2            # [32]

    l12 = np.zeros((KPI, 256), np.float16)
    l1 = l12[:, 0:128]
    l2xm = l12[:, 128:256]
    l2wm = np.zeros((128, 128), np.float16)
    l3 = np.zeros((128, 128), np.float16)
    l4 = np.zeros((128, KP), np.float16)
    for tl in range(TPC):
        # l1[6tl+d, 32tl+c] = A[c, d]; l1[24, 32tl+c] = c1[c]
        l1[6 * tl : 6 * tl + 6, 32 * tl : 32 * tl + 32] = A.T.astype(np.float16)
        l1[KP, 32 * tl : 32 * tl + 32] = c1.astype(np.float16)
        # l2x[6tl+d, 32tl+c] = A2x[c, d]; ones row carries c2x
        l2xm[6 * tl : 6 * tl + 6, 32 * tl : 32 * tl + 32] = A2x.T.astype(
            np.float16
        )
        l2xm[KP, 32 * tl : 32 * tl + 32] = c2x.astype(np.float16)
        # l2w[32tl+ci, 32tl+co] = W2[co, ci] / 2
        l2wm[32 * tl : 32 * tl + 32, 32 * tl : 32 * tl + 32] = (
            0.5 * W2.T
        ).astype(np.float16)
        # l3[32tl+co, 32tl+ci] = w3[co] * W2[co, ci]
        l3[32 * tl : 32 * tl + 32, 32 * tl : 32 * tl + 32] = (
            w3[:, None] * W2
        ).astype(np.float16)
        # l4[32tl+c, 6tl+d] = Bout[d, c]
        l4[32 * tl : 32 * tl + 32, 6 * tl : 6 * tl + 6] = Bout.T.astype(
            np.float16
        )

    return {
        "l12": l12,
        "l2w": l2wm,
        "l3": l3,
        "l4": l4,
    }


def _shard_x(x):
    """[B,S,N,D] -> per-core (fp16 [25, GROUPS] matmul layout with ones
    row, fp32 [64, OUTW*NB] quadrant-packed residual layout)."""
    xf = np.ascontiguousarray(np.asarray(x, np.float32)).reshape(TOK_TOTAL, D)
    shards = []
    for c in range(N_CORES):
        xc = xf[c * TOK_CORE : (c + 1) * TOK_CORE]          # [16384, 6]
        xp = xc.reshape(GROUPS, TPC, D).transpose(1, 2, 0).reshape(KP, GROUPS)
        xgc = np.empty((KPI, GROUPS), np.float16)
        xgc[:KP] = xp
        xgc[KP] = np.float16(1.0)
        xqc = np.zeros((64, OUTW * NB), np.float32)
        for bi, (c0, wd) in enumerate(BLOCKS):
            for h, off, cw in _chunks(wd):
                xqc[32 * h : 32 * h + KP, OUTW * bi : OUTW * bi + cw] = xp[
                    :, c0 + off : c0 + off + cw
                ]
        shards.append((xgc, xqc))
    return shards


def _unshard_out(outs):
    """list of per-core [64, OUTW*NB] -> [B,S,N,D].

    Block bi covers global cols c0..c0+wd; its 512-col chunk h sits on
    partitions 32h..32h+24 of outg[:, OUTW*bi : OUTW*bi+cw]."""
    full = np.empty((TOK_TOTAL, D), np.float32)
    for c, og in enumerate(outs):
        og = np.asarray(og)
        oc = np.empty((KP, GROUPS), np.float32)
        for bi, (c0, wd) in enumerate(BLOCKS):
            for h, off, cw in _chunks(wd):
                oc[:, c0 + off : c0 + off + cw] = og[
                    32 * h : 32 * h + KP, OUTW * bi : OUTW * bi + cw
                ]
        occ = oc.reshape(TPC, D, GROUPS).transpose(2, 0, 1).reshape(TOK_CORE, D)
        full[c * TOK_CORE : (c + 1) * TOK_CORE] = occ
    return full.reshape(B, S, N, D)


# Test-harness knobs (ignored in normal use): set kernel._TRACE = True to
# collect an NTFF profile; the BassKernelResults lands in kernel._LAST_RES.
_TRACE = False
_LAST_RES = None


def kernel(x, W_in, b_in, W1, b1, W2, b2, W3, b3):
    global _LAST_RES
    from concourse.bass_utils import run_bass_kernel_spmd

    nc = _get_nc()
    consts = _prep_weights(W_in, b_in, W1, b1, W2, b2, W3, b3)
    shards = _shard_x(x)
    in_maps = [
        {"xg": shards[c][0], "xq": shards[c][1], **consts}
        for c in range(N_CORES)
    ]
    res = run_bass_kernel_spmd(nc, in_maps, list(range(N_CORES)), trace=_TRACE)
    _LAST_RES = res
    return _unshard_out([res.results[c]["outg"] for c in range(N_CORES)])


# revision 5
# speedup vs baseline: 1.0709x; 1.0414x over previous
"""Trainium2 Bass kernel for nn_HamiltonianVersorNN.

Math: the reference energy reads only blade-0 of the final layer, and the
versor gate h*sigmoid(h[...,0:1]) makes blade-0 evolve as elementwise SiLU.
Backprop therefore collapses exactly to a 2-layer SiLU MLP on blade-0:

    a1 = A x + c1            A  = W1 @ W_in[:, ::32].T          [32, 6]
    a2 = W2 silu(a1) + c2    c1 = W1 @ b_in[::32] + b1[:, 0]
    dx = A.T (W2.T (w3 * silu'(a2)) * silu'(a1))
    out = x + dt * [dx[3:6], -dx[0:3]]

Sharding: pure data parallel over B*S*N positions, 8 cores, 16384
positions/core. On-chip layout packs 4 tokens per 128-partition column
(partition 32*tl + c holds channel c of token 4g+tl) so the 32x32
channel-mix matmuls contract over the full 128 partitions via
block-diagonal stationaries.

Perf design (v4, from 40.9us baseline):
- The PE streams at ~1.2 GHz on this part (512-col matmul ~427ns;
  dep-free spacing histogram shows nothing near the 2.4 GHz rate), so
  matmul passes over the 4096 columns dominate. Five passes are needed:
  l1 (a1 from x), l2x (a2 partial from x), l2w (a2 from w), l3 (v1 from
  d2), l4 (po from g1). The two x-consuming passes run as fp8e4m3
  DoubleRow (x and their stationaries laid out [13, 2, N]: contraction
  split in two k-tiles, 2 cols/cycle) - x only feeds the gradient path
  (the dt*grad term is ~1e-5 of the output scale), so fp8 there is
  harmless. Everything else is fp16: no fp32-HIGH mode anywhere, FWL
  stays available (fp32r movings silently run 4-pass and poison FWL).
- a2 = W2 silu(a1) + c2 is split as (W2/2)a1 + (W2/2)(a1 tanh(a1/2)):
  the (W2/2)a1 term composes with the input layer and streams straight
  from x; the only layer-1 elementwise product is w = a1*tau1 (DVE).
  Tanh and Derivative_silu share one ACT table set
  (derivative_silu_and_others) so there are zero table switches.
- The residual rides the mandatory PSUM->SBUF evacuation (DMA has no
  PSUM route): out = po + x as a DVE tensor_add against a host-prepared
  quadrant-packed fp32 copy of x - full precision, zero extra cost. po
  chunks are quadrant-packed by the l4 matmuls into partition strips
  (512-col chunk h -> partitions 32h..32h+24), halving the evacuation
  free-size on 1024 blocks.
- Queues: x + outputs on Sync (l12 first - it gates the first matmul),
  the merged back-half stationary block on Scalar (one DMA, done before
  the ACT table load), GpSimd entirely unused. Work tiles are allocated
  at uniform width under 6 tags so the Tile context's end-of-kernel
  semaphore drain stays short.
- Pipelining: back half split in two: (d2, l3) issue right after tau(k)
  so d2(k-1) takes the second ACT slot and l3(k-1) does not block the
  w-dependent l2w(k) on the PE queue; (g1, l4, evac, DMA) issue after
  l2w(k). Two small 256-col drain blocks shorten the final serial tail.
  PSUM: a1 double-buffered [2x2 banks], mid arena double-buffered
  [2x2 banks] rotating a2 -> v1 -> po.
"""

import sys

import numpy as np

if "/opt/trn_rl_repo" not in sys.path:
    sys.path.insert(0, "/opt/trn_rl_repo")

import concourse.bass as bass
import concourse.tile as tile
from concourse import mybir

AF = mybir.ActivationFunctionType
F32 = mybir.dt.float32
F16 = mybir.dt.float16
F8 = mybir.dt.float8e4
DR = mybir.MatmulPerfMode.DoubleRow

N_CORES = 8
B, S, N, D = 32, 256, 16, 6
HIDDEN = 32
BLADES = 32
DT = 0.01

TOK_TOTAL = B * S * N          # 131072 positions
TOK_CORE = TOK_TOTAL // N_CORES  # 16384
TPC = 4                        # tokens packed per 128-partition column
GROUPS = TOK_CORE // TPC       # 4096 columns per core
MM = 512                       # matmul free-dim (1 PSUM bank fp32)
BD = 1024                      # max block free-dim (2 PSUM banks)
BLOCKS = [(0, 512), (512, 1024), (1536, 1024), (2560, 1024),
          (3584, 256), (3840, 256)]
OUTW = 512                     # per-block HBM stride of the packed output
NB = len(BLOCKS)

KP = TPC * D                   # 24 partitions of x / out rows
KPI = KP + 1                   # + constant ones row carrying the biases
KT = 13                        # fp8 DoubleRow k-tile height (2*13 >= 26)


def _chunks(wd):
    """512-col matmul chunks of a block: (h, off, width)."""
    out = []
    h = 0
    off = 0
    while off < wd:
        out.append((h, off, min(MM, wd - off)))
        h += 1
        off += MM
    return out


def _build_nc():
    nc = bass.Bass()

    xg8 = nc.dram_tensor("xg8", [KT, 2, GROUPS], F8, kind="ExternalInput")
    xq = nc.dram_tensor("xq", [64, OUTW * NB], F32, kind="ExternalInput")
    l128 = nc.dram_tensor("l128", [KT, 2, 256], F8, kind="ExternalInput")
    cw = nc.dram_tensor("cw", [128, 280], F16, kind="ExternalInput")
    outg = nc.dram_tensor("outg", [64, OUTW * NB], F32, kind="ExternalOutput")

    with tile.TileContext(nc) as tc:
        with (
            tc.tile_pool(name="consts", bufs=1) as consts,
            tc.tile_pool(name="xin", bufs=1) as xin,
            tc.tile_pool(name="work", bufs=3) as work,
            tc.tile_pool(name="psA", bufs=2, space="PSUM") as psA,
            tc.tile_pool(name="psB", bufs=2, space="PSUM") as psB,
        ):
            # l128 gates the very first matmul, so it leads the Sync queue
            # (ahead of the x slabs). The merged back-half stationary block
            # rides Scalar (one DMA, configured before the ACT table load;
            # it is only needed one block in). GpSimd stays fully idle.
            sb_l128 = consts.tile([KT, 2, 256], F8)
            nc.sync.dma_start(out=sb_l128[:], in_=l128[:])
            sb_l1 = sb_l128[:, :, 0:128]
            sb_l2x = sb_l128[:, :, 128:256]
            sb_cw = consts.tile([128, 280], F16)
            nc.scalar.dma_start(out=sb_cw[:], in_=cw[:])
            sb_l2w = sb_cw[:, 0:128]
            sb_l3 = sb_cw[:, 128:256]
            sb_l4 = sb_cw[:, 256:280]

            # x arrives in two slabs (block 0's lands first), the
            # quadrant-packed fp32 residual copy in one.
            W0 = BLOCKS[0][1]
            sb_x8a = xin.tile([KT, 2, W0], F8)
            nc.sync.dma_start(out=sb_x8a[:], in_=xg8[:, :, 0:W0])
            sb_x8b = xin.tile([KT, 2, GROUPS - W0], F8)
            nc.sync.dma_start(out=sb_x8b[:], in_=xg8[:, :, W0:GROUPS])
            sb_xq = xin.tile([64, OUTW * NB], F32)
            nc.sync.dma_start(out=sb_xq[:], in_=xq[:])

            def xslab(bi, lo, hi):
                """SBUF view of x columns [lo, hi) of block bi."""
                c0 = BLOCKS[bi][0]
                if bi == 0:
                    return sb_x8a[:, :, c0 + lo : c0 + hi]
                return sb_x8b[:, :, c0 - W0 + lo : c0 - W0 + hi]

            # Dummy first activation: walrus attaches the ACT table load to
            # the first Activation instruction, which can then carry only a
            # single sync wait. Give it a single-wait warm-up op.
            warm = consts.tile([1, 128], F32)
            nc.vector.memset(warm[:], 0.0)
            nc.scalar.activation(warm[:], warm[:], AF.Derivative_silu)

            def back_early(st):
                """d2 -> v1 for the previous block. Issued right after
                tau(k) so d2(k-1) - whose input has been ready since last
                block - takes the second ACT slot, and l3(k-1) sits ahead
                of the w-dependent l2w(k) on the PE queue."""
                bi, wd, mid, d1 = st
                d2 = work.tile([128, BD], F16, tag="d2")
                nc.scalar.activation(d2[:, 0:wd], mid[:, 0:wd],
                                     AF.Derivative_silu)
                for h, off, cw_ in _chunks(wd):
                    ms = slice(off, off + cw_)
                    nc.tensor.matmul(mid[:, ms], sb_l3, d2[:, ms],
                                     start=True, stop=True)
                return d2

            def back_late(st):
                """g1 -> po (quadrant-packed) -> evac(+residual) -> DMA."""
                bi, wd, mid, d1 = st
                g1 = work.tile([128, BD], F16, tag="g1")
                nc.vector.tensor_mul(g1[:, 0:wd], mid[:, 0:wd], d1[:, 0:wd])

                # po = blockdiag(Bout) @ g1, chunk h on partitions
                # 32h..32h+24 (overwrites v1, dead after g1).
                nch = 0
                for h, off, cw_ in _chunks(wd):
                    po = mid[32 * h : 32 * h + KP, 0:cw_]
                    nc.tensor.matmul(po, sb_l4, g1[:, off : off + cw_],
                                     start=True, stop=True)
                    nch += 1

                # Mandatory PSUM->SBUF evacuation doubles as the residual
                # add against the quadrant-packed fp32 x.
                ew = min(wd, MM)
                sb_o = work.tile([64, MM], F32, tag="o")
                nc.vector.tensor_add(
                    sb_o[0 : 32 * nch, 0:ew],
                    mid[0 : 32 * nch, 0:ew],
                    sb_xq[0 : 32 * nch, OUTW * bi : OUTW * bi + ew],
                )
                nc.sync.dma_start(
                    out=outg[0 : 32 * nch, OUTW * bi : OUTW * bi + ew],
                    in_=sb_o[0 : 32 * nch, 0:ew],
                )

            pending = None
            for bi, (c0, wd) in enumerate(BLOCKS):
                # a1 = blockdiag(A) @ x + c1 (c1 rides the ones row);
                # fp8 DoubleRow: contraction over 2 k-tiles of 13 rows.
                a1 = psA.tile([128, BD], F32, tag="a1")
                for h, off, cw_ in _chunks(wd):
                    nc.tensor.matmul(a1[:, off : off + cw_], sb_l1,
                                     xslab(bi, off, off + cw_),
                                     start=True, stop=True, perf_mode=DR)

                # a2 = W2 silu(a1) + c2 split as (W2/2) a1 + (W2/2)(a1*tau):
                # the first term composes with the input layer, so it
                # streams straight from x (l2x = blockdiag(W2 A / 2) with
                # bias row W2 c1 / 2 + c2).
                mid = psB.tile([128, BD], F32, tag="mid")
                for h, off, cw_ in _chunks(wd):
                    nc.tensor.matmul(mid[:, off : off + cw_], sb_l2x,
                                     xslab(bi, off, off + cw_),
                                     start=True, stop=False, perf_mode=DR)

                # tau = tanh(0.5*a1)
                tau = work.tile([128, BD], F16, tag="tau")
                nc.scalar.activation(tau[:, 0:wd], a1[:, 0:wd], AF.Tanh,
                                     scale=0.5)

                # w = a1 * tau; issued now so it leads the DVE queue.
                w = work.tile([128, BD], F16, tag="w")
                nc.vector.tensor_mul(w[:, 0:wd], a1[:, 0:wd], tau[:, 0:wd])

                if pending is not None:
                    back_early(pending)

                # d1 = silu'(a1) (consumer g1 is a block away, so d2(k-1)
                # above takes the second ACT slot)
                d1 = work.tile([128, BD], F16, tag="d1")
                nc.scalar.activation(d1[:, 0:wd], a1[:, 0:wd],
                                     AF.Derivative_silu)

                # a2 += blockdiag(W2/2) @ w
                for h, off, cw_ in _chunks(wd):
                    ms = slice(off, off + cw_)
                    nc.tensor.matmul(mid[:, ms], sb_l2w, w[:, ms],
                                     start=False, stop=True)

                if pending is not None:
                    back_late(pending)

                pending = (bi, wd, mid, d1)

            back_early(pending)
            back_late(pending)

    return nc


def _split_multi_waits(nc):
    """This walrus build rejects engine instructions carrying more than one
    sync wait ("Too many sync wait commands"). Hoist all but one wait of
    each instruction onto standalone NoOps issued just before it on the
    same engine (engines execute their queue in order, so semantics are
    preserved)."""
    for f in nc.m.functions:
        for b in f.blocks:
            insts = list(b.instructions)
            out = []
            changed = False
            for inst in insts:
                # This walrus build also rejects the raw-ISA
                # EVENT_SEMAPHORE_RANGE_CLEAR Tile emits at context end
                # ("ISA wrong length" - ISA table version skew). The NEFF
                # preamble re-initializes semaphores, so drop it.
                if (
                    type(inst).__name__ == "InstISA"
                    and getattr(inst, "op_name", "") == "EVENT_SEMAPHORE_RANGE_CLEAR"
                ):
                    changed = True
                    continue
                si = getattr(inst, "sync_info", None)
                waits = list(si.on_wait) if si is not None and si.on_wait else []
                if len(waits) > 1:
                    changed = True
                    for k, w in enumerate(waits[:-1]):
                        nop = mybir.InstNoOp(name=f"{inst.name}-w{k}", ins=[], outs=[])
                        nop.engine = inst.engine
                        nop.sync_info = mybir.SyncInfo(on_wait=[w], on_update=[])
                        out.append(nop)
                    inst.sync_info = mybir.SyncInfo(
                        on_wait=[waits[-1]], on_update=list(si.on_update or [])
                    )
                out.append(inst)
            if changed:
                b.instructions = out
    return nc


_NC_CACHE = None


def _get_nc():
    global _NC_CACHE
    if _NC_CACHE is None:
        _NC_CACHE = _split_multi_waits(_build_nc())
    return _NC_CACHE


def _fold_kt(rows):
    """[26, M] -> [13, 2, M] DoubleRow k-tile layout (row = 13*ko + ki)."""
    return rows.reshape(2, KT, -1).transpose(1, 0, 2)


def _prep_weights(W_in, b_in, W1, b1, W2, b2, W3, b3):
    """Host-side constant folding into the kernel's stationary layouts."""
    import ml_dtypes

    F8NP = ml_dtypes.float8_e4m3

    W_in = np.asarray(W_in, np.float64)
    b_in = np.asarray(b_in, np.float64)
    W1 = np.asarray(W1, np.float64)
    b1 = np.asarray(b1, np.float64)
    W2 = np.asarray(W2, np.float64)
    b2 = np.asarray(b2, np.float64)
    W3 = np.asarray(W3, np.float64)

    Win0 = W_in[:, ::BLADES]            # [6, 8]
    bin0 = b_in[::BLADES]               # [8]
    A = W1 @ Win0.T                     # [32, 6]
    c1 = W1 @ bin0 + b1[:, 0]           # [32]
    c2 = b2[:, 0]                       # [32]
    w3 = W3[0, :]                       # [32]

    # Bout[d, c]: out[d] += dt*dx[d+3] (d<3), -dt*dx[d-3] (d>=3); dx = A^T g1
    Bout = np.zeros((D, HIDDEN))
    Bout[0:3, :] = DT * A[:, 3:6].T
    Bout[3:6, :] = -DT * A[:, 0:3].T

    # a2 = W2 silu(a1) + c2 = (W2/2) a1 + (W2/2)(a1 tau1) + c2, and
    # (W2/2) a1 = (W2 A / 2) x + W2 c1 / 2  composes with the input layer.
    A2x = 0.5 * W2 @ A                  # [32, 6]
    c2x = 0.5 * W2 @ c1 + c2            # [32]

    l12 = np.zeros((2 * KT, 256), np.float32)
    l1 = l12[:, 0:128]
    l2xm = l12[:, 128:256]
    cwm = np.zeros((128, 280), np.float16)
    l2wm = cwm[:, 0:128]
    l3 = cwm[:, 128:256]
    l4 = cwm[:, 256:280]
    for tl in range(TPC):
        # l1[6tl+d, 32tl+c] = A[c, d]; l1[24, 32tl+c] = c1[c]
        l1[6 * tl : 6 * tl + 6, 32 * tl : 32 * tl + 32] = A.T
        l1[KP, 32 * tl : 32 * tl + 32] = c1
        # l2x[6tl+d, 32tl+c] = A2x[c, d]; ones row carries c2x
        l2xm[6 * tl : 6 * tl + 6, 32 * tl : 32 * tl + 32] = A2x.T
        l2xm[KP, 32 * tl : 32 * tl + 32] = c2x
        # l2w[32tl+ci, 32tl+co] = W2[co, ci] / 2
        l2wm[32 * tl : 32 * tl + 32, 32 * tl : 32 * tl + 32] = (
            0.5 * W2.T
        ).astype(np.float16)
        # l3[32tl+co, 32tl+ci] = w3[co] * W2[co, ci]
        l3[32 * tl : 32 * tl + 32, 32 * tl : 32 * tl + 32] = (
            w3[:, None] * W2
        ).astype(np.float16)
        # l4[32tl+c, 6tl+d] = Bout[d, c]
        l4[32 * tl : 32 * tl + 32, 6 * tl : 6 * tl + 6] = Bout.T.astype(
            np.float16
        )

    return {
        "l128": np.ascontiguousarray(_fold_kt(l12)).astype(F8NP),
        "cw": cwm,
    }


def _shard_x(x):
    """[B,S,N,D] -> per-core (fp8 [13, 2, GROUPS] DoubleRow matmul layout
    with ones row, fp32 [64, OUTW*NB] quadrant-packed residual layout)."""
    import ml_dtypes

    F8NP = ml_dtypes.float8_e4m3

    xf = np.ascontiguousarray(np.asarray(x, np.float32)).reshape(TOK_TOTAL, D)
    shards = []
    for c in range(N_CORES):
        xc = xf[c * TOK_CORE : (c + 1) * TOK_CORE]          # [16384, 6]
        xp = xc.reshape(GROUPS, TPC, D).transpose(1, 2, 0).reshape(KP, GROUPS)
        xe = np.zeros((2 * KT, GROUPS), np.float32)
        xe[:KP] = xp
        xe[KP] = 1.0
        x8 = np.ascontiguousarray(_fold_kt(xe)).astype(F8NP)
        xqc = np.zeros((64, OUTW * NB), np.float32)
        for bi, (c0, wd) in enumerate(BLOCKS):
            for h, off, cw_ in _chunks(wd):
                xqc[32 * h : 32 * h + KP, OUTW * bi : OUTW * bi + cw_] = xp[
                    :, c0 + off : c0 + off + cw_
                ]
        shards.append((x8, xqc))
    return shards


def _unshard_out(outs):
    """list of per-core [64, OUTW*NB] -> [B,S,N,D].

    Block bi covers global cols c0..c0+wd; its 512-col chunk h sits on
    partitions 32h..32h+24 of outg[:, OUTW*bi : OUTW*bi+cw]."""
    full = np.empty((TOK_TOTAL, D), np.float32)
    for c, og in enumerate(outs):
        og = np.asarray(og)
        oc = np.empty((KP, GROUPS), np.float32)
        for bi, (c0, wd) in enumerate(BLOCKS):
            for h, off, cw_ in _chunks(wd):
                oc[:, c0 + off : c0 + off + cw_] = og[
                    32 * h : 32 * h + KP, OUTW * bi : OUTW * bi + cw_
                ]
        occ = oc.reshape(TPC, D, GROUPS).transpose(2, 0, 1).reshape(TOK_CORE, D)
        full[c * TOK_CORE : (c + 1) * TOK_CORE] = occ
    return full.reshape(B, S, N, D)


# Test-harness knobs (ignored in normal use): set kernel._TRACE = True to
# collect an NTFF profile; the BassKernelResults lands in kernel._LAST_RES.
_TRACE = False
_LAST_RES = None


def kernel(x, W_in, b_in, W1, b1, W2, b2, W3, b3):
    global _LAST_RES
    from concourse.bass_utils import run_bass_kernel_spmd

    nc = _get_nc()
    consts = _prep_weights(W_in, b_in, W1, b1, W2, b2, W3, b3)
    shards = _shard_x(x)
    in_maps = [
        {"xg8": shards[c][0], "xq": shards[c][1], **consts}
        for c in range(N_CORES)
    ]
    res = run_bass_kernel_spmd(nc, in_maps, list(range(N_CORES)), trace=_TRACE)
    _LAST_RES = res
    return _unshard_out([res.results[c]["outg"] for c in range(N_CORES)])
